# revision 9
# baseline (speedup 1.0000x reference)
"""Multi-head attention (nn_Attention_18528488915211) on 8 Trainium2 NeuronCores.

Sharding: tensor-parallel over heads. 16 heads / 8 cores = 2 heads per core.
Each core computes Q/K/V projections for its 256 columns of Wq/Wk/Wv,
attention for its 2 heads, and a partial output projection with its 256 rows
of Wo. The host sums the 8 partial outputs (the TP all-reduce) and adds bo.

v3 design (fp16/bf16 datapath, PE-bound software-pipelined schedule):
  - All 16-bit data; every matmul is [128,128] stationary x [128,512] moving
    at 1 cycle/row, so LDWEIGHTS (107ns) hides behind each 213ns matmul.
  - Q^T/K^T/V^T projections weights-stationary; V natural layout for AV is
    produced by the DMA XBAR transpose (SBUF->SBUF), costing no engine time.
  - Attention per 512-query chunk (ic), heads interleaved, AV pipelined one
    key-block behind the Scalar-engine exp; output-projection matmuls of the
    previous chunk fill PE slack from slot 4 on (slots 0-3 cover the chunk's
    rowsum/normalize tail emitted at slot 0).
  - Each chunk's tail (last AV, ones-matmul rowsum, reciprocal, normalize)
    is deferred past the next chunk's first S-pair so the Scalar engine
    never drains; the last chunk of a batch defers its tail into the next
    batch's first projection group.
  - xt tiles for batch b+1 prefetch chunk-by-chunk at each B(b) chunk start
    so output-tile DMAs never queue behind a 4MB burst.
  - PSUM banks: st ring 2 + ot_h0/h1 2x2 + shared proj/out/rowsum ring 2 = 8.
"""

import numpy as np

P = 128          # partitions
DM = 2048        # dmodel
DH = 128         # dhead
HPC = 2          # heads per core
DC = HPC * DH    # dmodel columns per core (256)
B = 4            # batch
L = 2048         # sequence length
T = B * L        # total tokens (8192)
KS = DM // P     # contraction subtiles (16)
TC = 512         # token/query chunk (matmul moving dim)
NCH = L // TC    # chunks per batch (4)
NJ = L // P      # key blocks per batch (16)
NCORES = 8


def _build_nc():
    import concourse.mybir as mybir
    import concourse.tile as tile
    from concourse import bacc

    f32 = mybir.dt.float32
    f16 = mybir.dt.float16
    bf16 = mybir.dt.bfloat16
    EXP = mybir.ActivationFunctionType.Exp

    nc = bacc.Bacc("TRN2", target_bir_lowering=False, debug=False,
                   num_devices=NCORES)

    xt = nc.dram_tensor("xt", [DM, T], f16, kind="ExternalInput").ap()
    wq = nc.dram_tensor("wq", [DM, DC], f16, kind="ExternalInput").ap()
    wk = nc.dram_tensor("wk", [DM, DC], f16, kind="ExternalInput").ap()
    wv = nc.dram_tensor("wv", [DM, DC], f16, kind="ExternalInput").ap()
    bq = nc.dram_tensor("bq", [DC], f32, kind="ExternalInput").ap()
    bk = nc.dram_tensor("bk", [DC], f32, kind="ExternalInput").ap()
    bv = nc.dram_tensor("bv", [DC], f32, kind="ExternalInput").ap()
    wo = nc.dram_tensor("wo", [DC, DM], f16, kind="ExternalInput").ap()
    out = nc.dram_tensor("out", [T, DM], f16, kind="ExternalOutput").ap()

    with tile.TileContext(nc) as tc:
        with (
            tc.tile_pool(name="wpool", bufs=1) as wpool,
            tc.tile_pool(name="xpool", bufs=32) as xpool,
            tc.tile_pool(name="qkv", bufs=2) as qkv,
            tc.tile_pool(name="misc", bufs=2) as misc,
            tc.tile_pool(name="psum", bufs=2, space="PSUM") as psum,
        ):
            xt_cache = {}

            def load_chunk(b, c):
                if (b, c) in xt_cache:
                    return xt_cache.pop((b, c))
                return _claim(b, c)

            def prefetch_chunk(b, c):
                if b < B and (b, c) not in xt_cache:
                    xt_cache[(b, c)] = _claim(b, c)

            def _claim(b, c):
                tiles = []
                for ks in range(KS):
                    xt_t = xpool.tile([P, TC], f16, tag="xt")
                    nc.sync.dma_start(
                        xt_t[:],
                        xt[ks * P:(ks + 1) * P,
                           b * L + c * TC: b * L + (c + 1) * TC],
                    )
                    tiles.append(xt_t)
                return tiles

            # --- resident weights (ordered so the first projection group
            # can start as early as possible) ---
            wq_sb = wpool.tile([P, KS, DC], f16, tag="wq")
            wk_sb = wpool.tile([P, KS, DC], f16, tag="wk")
            wv_sb = wpool.tile([P, KS, DC], f16, tag="wv")
            bq_sb = wpool.tile([P, HPC], f32, tag="bq")
            bk_sb = wpool.tile([P, HPC], f32, tag="bk")
            bv_sb = wpool.tile([P, HPC], f32, tag="bv")
            for ks in range(KS):
                nc.sync.dma_start(wq_sb[:, ks, :], wq[ks * P:(ks + 1) * P, :])
            nc.sync.dma_start(bq_sb[:], bq.rearrange("(h d) -> d h", d=P))
            xt_cache[(0, 0)] = _claim(0, 0)
            for ks in range(KS):
                nc.sync.dma_start(wk_sb[:, ks, :], wk[ks * P:(ks + 1) * P, :])
            nc.sync.dma_start(bk_sb[:], bk.rearrange("(h d) -> d h", d=P))
            xt_cache[(0, 1)] = _claim(0, 1)
            for ks in range(KS):
                nc.sync.dma_start(wv_sb[:, ks, :], wv[ks * P:(ks + 1) * P, :])
            nc.sync.dma_start(bv_sb[:], bv.rearrange("(h d) -> d h", d=P))
            ones_sb = wpool.tile([P, P], bf16, tag="ones")
            nc.any.memset(ones_sb[:], 1.0)
            wo_sb = wpool.tile([P, HPC, DM], f16, tag="wo")
            nc.sync.dma_start(wo_sb[:], wo.rearrange("(h p) n -> p h n", p=P))

            # Output-projection work for one finished 512-token chunk,
            # emitted 1 matmul per yield (pumped as PE filler work).
            def o_work_gen(ot_sb, qoff, t0):
                for tb in range(TC // P):
                    tsl = slice(qoff + tb * P, qoff + (tb + 1) * P)
                    for ncl in range(DM // TC):
                        o_ps = psum.tile([P, TC], f32, tag="ps", name="o_ps")
                        nc.tensor.matmul(
                            o_ps[:], ot_sb[:, 0, tsl],
                            wo_sb[:, 0, ncl * TC:(ncl + 1) * TC],
                            start=True, stop=False,
                        )
                        yield
                        nc.tensor.matmul(
                            o_ps[:], ot_sb[:, 1, tsl],
                            wo_sb[:, 1, ncl * TC:(ncl + 1) * TC],
                            start=False, stop=True,
                        )
                        o_sb = misc.tile([P, TC], f16, tag="oout",
                                         name="o_sb", bufs=6)
                        nc.vector.tensor_copy(o_sb[:], o_ps[:])
                        nc.sync.dma_start(
                            out[t0 + tb * P: t0 + (tb + 1) * P,
                                ncl * TC:(ncl + 1) * TC],
                            o_sb[:],
                        )
                        yield

            o_gens = []

            def pump(n):
                while n > 0 and o_gens:
                    try:
                        next(o_gens[0])
                        n -= 1
                    except StopIteration:
                        o_gens.pop(0)

            # Deferred per-chunk tail: last AV pair, rowsum, recip, normalize.
            pending_tail = [None]

            def run_tail():
                if pending_tail[0] is not None:
                    t, pending_tail[0] = pending_tail[0], None
                    t()

            def make_tail(vn, otp, racc, pt_m2, pt_m1, ot, qs, qoff, t0):
                def tail():
                    for h in range(HPC):
                        nc.tensor.matmul(otp[:, h, :], vn[:, NJ - 2, h, :],
                                         pt_m2[:, h, :], start=False,
                                         stop=False, skip_group_check=True)
                    for h in range(HPC):
                        nc.tensor.matmul(otp[:, h, :], vn[:, NJ - 1, h, :],
                                         pt_m1[:, h, :], start=False,
                                         stop=True, skip_group_check=True)
                    # release the ot PSUM tile fast: drain unnormalized
                    ot_u = misc.tile([P, HPC, TC], f16, tag="otu",
                                     name="ot_u")
                    nc.vector.tensor_copy(ot_u[:], otp[:])
                    # off the critical path: rowsums, reciprocal, normalize
                    for h in range(HPC):
                        rs = psum.tile([P, TC], f32, tag="ps", name="rs")
                        nc.tensor.matmul(rs[:], ones_sb[:], racc[:, h, :],
                                         start=True, stop=True)
                        rcp = misc.tile([P, TC], f32, tag="rcp", name="rcp")
                        nc.vector.reciprocal_approx_fast(rcp[:], rs[:])
                        nc.vector.tensor_mul(ot[:, h, qs], ot_u[:, h, :],
                                             rcp[:])
                    o_gens.append(o_work_gen(ot, qoff, t0))
                return tail

            for b in range(B):
                t0 = b * L
                qt = qkv.tile([P, HPC, L], f16, tag="qt", name="qt")
                kt = qkv.tile([P, HPC, L], f16, tag="kt", name="kt")
                vt = qkv.tile([P, HPC, L], bf16, tag="vt", name="vt")
                vn = qkv.tile([P, NJ, HPC, DH], bf16, tag="vn", name="vn")
                ot = qkv.tile([P, HPC, L], f16, tag="ot", name="ot")

                # ============ Phase A: Q^T/K^T/V^T projections ============
                for c in range(NCH):
                    cs = slice(c * TC, (c + 1) * TC)
                    xts = load_chunk(b, c)
                    for w_sb, b_sb, dest in ((wq_sb, bq_sb, qt),
                                             (wk_sb, bk_sb, kt),
                                             (wv_sb, bv_sb, vt)):
                        for h in range(HPC):
                            acc = psum.tile([P, TC], f32, tag="ps",
                                            name="proj")
                            for ks in range(KS):
                                nc.tensor.matmul(
                                    acc[:],
                                    w_sb[:, ks, h * DH:(h + 1) * DH],
                                    xts[ks][:],
                                    start=(ks == 0), stop=(ks == KS - 1),
                                )
                            nc.vector.tensor_scalar_add(
                                dest[:, h, cs], acc[:], b_sb[:, h:h + 1])
                            # previous batch's last-chunk tail rides behind
                            # the first projection group of this batch
                            run_tail()
                    for h in range(HPC):
                        nc.sync.dma_start_transpose(
                            vn[:, c * (TC // P):(c + 1) * (TC // P), h, :],
                            vt[:, h, cs],
                        )

                # ===== Phase B: attention, with fused output projection =====
                for ic in range(NCH):
                    if b + 1 < B:
                        prefetch_chunk(b + 1, ic)
                    qs = slice(ic * TC, (ic + 1) * TC)
                    otp = psum.tile([P, HPC, TC], f32, tag="ot", name="otp",
                                    bufs=1)
                    racc = misc.tile([P, HPC, TC], bf16, tag="racc",
                                     name="racc")
                    pts = []
                    for js in range(NJ):
                        ksl = slice(js * P, (js + 1) * P)
                        st = psum.tile([P, HPC, TC], f32, tag="st", name="st")
                        for h in range(HPC):
                            nc.tensor.matmul(st[:, h, :], kt[:, h, ksl],
                                             qt[:, h, qs], start=True,
                                             stop=True, skip_group_check=True)
                        if js == 0:
                            run_tail()  # previous chunk's tail
                        pt = misc.tile([P, HPC, TC], bf16, tag="pt",
                                       name="pt", bufs=4)
                        nc.scalar.activation(pt[:], st[:], EXP,
                                             scale=1.0 / DH)
                        if js == 1:
                            nc.vector.tensor_add(racc[:, 0, :],
                                                 pts[0][:, 0, :],
                                                 pt[:, 0, :])
                            nc.gpsimd.tensor_add(racc[:, 1, :],
                                                 pts[0][:, 1, :],
                                                 pt[:, 1, :])
                        elif js > 1:
                            nc.vector.tensor_add(racc[:, 0, :], racc[:, 0, :],
                                                 pt[:, 0, :])
                            nc.gpsimd.tensor_add(racc[:, 1, :], racc[:, 1, :],
                                                 pt[:, 1, :])
                        if js >= 2:
                            for h in range(HPC):
                                nc.tensor.matmul(
                                    otp[:, h, :], vn[:, js - 2, h, :],
                                    pts[js - 2][:, h, :], start=(js == 2),
                                    stop=False, skip_group_check=True)
                        if js >= 4:
                            pump(3)
                        pts.append(pt)
                    pending_tail[0] = make_tail(vn, otp, racc, pts[NJ - 2],
                                                pts[NJ - 1], ot, qs, ic * TC,
                                                t0 + ic * TC)

            # final flush: last chunk's tail + remaining output projection
            run_tail()
            pump(1 << 30)

    nc.compile()
    return nc


_NC_CACHE = None


def kernel(**inputs: np.ndarray) -> np.ndarray:
    from concourse.bass_utils import run_bass_kernel_spmd

    global _NC_CACHE
    f16 = np.float16
    x = np.asarray(inputs["x"], dtype=np.float32)
    Wq, bq = np.asarray(inputs["Wq"]), np.asarray(inputs["bq"])
    Wk, bk = np.asarray(inputs["Wk"]), np.asarray(inputs["bk"])
    Wv, bv = np.asarray(inputs["Wv"]), np.asarray(inputs["bv"])
    Wo, bo = np.asarray(inputs["Wo"]), np.asarray(inputs["bo"])

    xt = np.ascontiguousarray(x.reshape(T, DM).T).astype(f16)

    in_maps = []
    for c in range(NCORES):
        sl = slice(c * DC, (c + 1) * DC)
        in_maps.append({
            "xt": xt,
            "wq": np.ascontiguousarray(Wq[:, sl]).astype(f16),
            "wk": np.ascontiguousarray(Wk[:, sl]).astype(f16),
            "wv": np.ascontiguousarray(Wv[:, sl]).astype(f16),
            "bq": np.ascontiguousarray(bq[sl]).astype(np.float32),
            "bk": np.ascontiguousarray(bk[sl]).astype(np.float32),
            "bv": np.ascontiguousarray(bv[sl]).astype(np.float32),
            "wo": np.ascontiguousarray(Wo[sl, :]).astype(f16),
        })

    if _NC_CACHE is None:
        _NC_CACHE = _build_nc()
    res = run_bass_kernel_spmd(_NC_CACHE, in_maps, core_ids=list(range(NCORES)))

    acc = res.results[0]["out"].astype(np.float32)
    for c in range(1, NCORES):
        acc = acc + res.results[c]["out"].astype(np.float32)
    acc = acc + bo[None, :].astype(np.float32)
    return acc.reshape(B, L, DM)
